# revision 21
# baseline (speedup 1.0000x reference)
"""GAT 2-layer node classifier on 8 Trainium2 NeuronCores.

Strategy (1D node partitioning + batched SWDGE gathers):
  - dst nodes sharded contiguously across 8 cores (12500 each)
  - host: per core, group 128 dst nodes per "group" (band+argmax profile
    grouping to minimize padding), in-edges packed along the free dim,
    split by 4 source windows (shard pairs, <=25088 rows so dma_gather's
    int16 indices can address them); padded slots point to a pad row
    whose attention logit is -3e38 so exp() contributes 0.
  - device per layer: one dma_gather per (group, window) pulls all edge
    rows (256B each) in a single SWDGE instruction (the previous
    per-edge-slot indirect DMAs paid ~1us of descriptor-generation
    overhead each and dominated the runtime), then softmax-weighted
    aggregation on DVE/Act, layer-2 projection fused into the layer-1
    loop, AllGather of the per-shard gather table between layers.
  - layer-1 table rows: [feat64(bf16, d-major) | ones8(bf16) | el8(f32)]
    = 256B; layer-2 rows: [feat40(f32) | el(f32) | pad] = 256B.
"""

import sys
import types

import numpy as np
import ml_dtypes

# ---------------------------------------------------------------------------
# environment shims (self-contained: only touches in-process state)
# ---------------------------------------------------------------------------


def _ensure_axon_hooks():
    """concourse.bass_utils imports antenv.axon_hooks when tracing under
    axon; some images lack the module. Provide an in-process shim."""
    try:
        import antenv.axon_hooks  # noqa: F401
        return
    except Exception:
        pass
    try:
        import antenv
    except Exception:
        return
    mod = types.ModuleType("antenv.axon_hooks")
    mod._hook = None

    def set_axon_ntff_profile_hook(hook):
        mod._hook = hook

    def get_axon_ntff_profile_hook():
        return mod._hook

    mod.set_axon_ntff_profile_hook = set_axon_ntff_profile_hook
    mod.get_axon_ntff_profile_hook = get_axon_ntff_profile_hook
    sys.modules["antenv.axon_hooks"] = mod
    antenv.axon_hooks = mod


_ensure_axon_hooks()

import concourse.bass as bass          # noqa: E402
import concourse.mybir as mybir        # noqa: E402
import concourse.tile as tile          # noqa: E402
from concourse import library_config   # noqa: E402
from concourse.vector_clock import ScopedClock  # noqa: E402
from concourse.bass_utils import run_bass_kernel_spmd  # noqa: E402

F32 = mybir.dt.float32
BF16 = mybir.dt.bfloat16
I16 = mybir.dt.int16
AF = mybir.ActivationFunctionType
OP = mybir.AluOpType
AX = mybir.AxisListType


def _patched_drain_and_barrier(self, tick_clock, wait_clock):
    # this walrus build rejects multi-wait instructions; emit one wait per
    # nop before the tail drain instead of stacking them on the drain.
    nc = self.nc
    probe = nc.sync.nop(nofuse=True)
    wait_clock.add_sem_waits(probe.ins, ScopedClock({None: tick_clock.global_clock}))
    waits = list(probe.ins.sync_info.on_wait or []) if probe.ins.sync_info else []
    if waits:
        probe.ins.sync_info = mybir.SyncInfo(on_wait=[waits[0]], on_update=[])
        for w in waits[1:]:
            nop = nc.sync.nop(nofuse=True)
            nop.ins.sync_info = mybir.SyncInfo(on_wait=[w], on_update=[])
    nc.sync.drain()
    nc.all_engine_barrier()
    popped = nc._tile_sem_poison_stack.pop()
    assert popped is self._sem_poison
    nc.clear_and_free_semaphores(list(self.sems.allocated().values()))
    nc.all_engine_barrier()


tile.TileContext._drain_and_barrier = _patched_drain_and_barrier


def _split_waits(nc, max_waits=1):
    """Post-pass: any instruction carrying more than max_waits sem-waits gets
    preceding same-engine NoOps carrying the excess."""
    uid = [0]
    for f in nc.m.functions:
        for bb in f.blocks:
            new_insts = []
            for inst in bb.instructions:
                si = getattr(inst, "sync_info", None)
                if si is not None and si.on_wait and len(si.on_wait) > max_waits:
                    waits = list(si.on_wait)
                    excess, keep = waits[:-max_waits], waits[-max_waits:]
                    for i in range(0, len(excess), max_waits):
                        uid[0] += 1
                        new_insts.append(mybir.InstNoOp(
                            name=f"waitsplit-{uid[0]}-{inst.name}",
                            sync_info=mybir.SyncInfo(
                                on_wait=excess[i:i + max_waits], on_update=[]),
                            bass_nofuse=True,
                            engine=inst.engine,
                        ))
                    inst.sync_info = mybir.SyncInfo(
                        on_wait=keep, on_update=list(si.on_update or []))
                new_insts.append(inst)
            bb.instructions = new_insts


# ---------------------------------------------------------------------------
# problem constants (hardcoded per spec)
# ---------------------------------------------------------------------------
N_CORES = 8
V = 100000            # nodes
S = V // N_CORES      # nodes per core shard (12500)
F_IN = 256
H1, D1 = 8, 8         # layer-1 heads x dim
HD1 = H1 * D1         # 64
C2 = 40               # classes (layer-2 single head)
NEG_SLOPE = 0.2
NEG_BIG = -1.0e30
G = (S + 127) // 128  # 98 groups of 128 dst nodes per core
SPT = G * 128         # 12544 table rows per shard
NW = 4                # source windows (shard pairs)
WROWS = 2 * SPT       # 25088 rows per window (< 32768: int16-indexable)
DUMMY_LOC = S         # window-local dummy row (= first pad row of shard 2w)
ROW1 = 128            # layer-1 table row: bf16[128] = 256B
ROW2 = 64             # layer-2 table row: f32[64]  = 256B

# module-level knobs (test harness pokes these)
PROFILE = False
DEBUG = False
LAST_EXEC_NS = None
LAST_RESULTS = None


# ---------------------------------------------------------------------------
# host-side graph preprocessing (integer work only)
# ---------------------------------------------------------------------------

def _host_prep(src, dst):
    src = np.asarray(src).astype(np.int64)
    dst = np.asarray(dst).astype(np.int64)
    order = np.argsort(dst, kind="stable")
    src_s = src[order].astype(np.int64)
    dst_s = dst[order].astype(np.int64)
    bounds = np.searchsorted(dst_s, np.arange(N_CORES + 1) * S)
    wid_all = (src_s // S) >> 1          # source window of each edge

    # per-core grouping: band+argmax profile sort to minimize per-window
    # max-over-partition padding
    perms, cws = [], []
    for c in range(N_CORES):
        sl = slice(bounds[c], bounds[c + 1])
        ldst = dst_s[sl] - c * S
        cw = np.zeros((S, NW), np.int64)
        np.add.at(cw, (ldst, wid_all[sl]), 1)
        deg = cw.sum(1)
        o = np.argsort(-deg, kind="stable")
        band = 1024
        ob = o.copy()
        for b in range(0, S, band):
            idx = o[b:b + band]
            sub = cw[idx]
            amx = np.argmax(sub, 1)
            key2 = -sub[np.arange(len(idx)), amx]
            ob[b:b + band] = idx[np.lexsort((key2, amx))]
        perms.append(ob.astype(np.int64))
        cws.append(cw)

    # table position of every node (within its shard) under final grouping
    pos_all = np.empty(V, np.int64)
    for c in range(N_CORES):
        pos = np.empty(S, np.int64)
        pos[perms[c]] = np.arange(S)
        pos_all[c * S:(c + 1) * S] = pos
    # window-local table row of every node
    loc_row = ((np.arange(V) // S) % 2) * SPT + pos_all   # < 25088

    # shared per-(group, window) column counts: max over cores
    Lgw_c = []
    for c in range(N_CORES):
        cw_perm = cws[c][perms[c]]
        cw_pad = np.vstack([cw_perm, np.zeros((SPT - S, NW), np.int64)])
        Lgw_c.append(cw_pad.reshape(G, 128, NW).max(1))
    Lgw = np.maximum(np.max(np.stack(Lgw_c), axis=0), 1)   # [G, NW], >=1
    # quantize column counts so the distinct set of 128*L immediates fits in
    # the ~48 free Pool registers (to_reg allocates one per distinct value)
    Lgw = np.where(Lgw <= 16, Lgw, ((Lgw + 3) // 4) * 4)
    offw = np.concatenate(
        [np.zeros((G, 1), np.int64), np.cumsum(Lgw, 1)], 1)  # [G, NW+1]
    Ltot = offw[:, -1]
    gbase = np.concatenate([[0], np.cumsum(Ltot * 128)]).astype(np.int64)
    TOT = int(gbase[-1])

    # per-core int16 index arrays in dma_gather wrapped layout
    idx_tiles = []
    for c in range(N_CORES):
        sl = slice(bounds[c], bounds[c + 1])
        ldst = dst_s[sl] - c * S
        esrc = src_s[sl]
        ew = wid_all[sl]
        posn = np.empty(S, np.int64)
        posn[perms[c]] = np.arange(S)
        p_of = posn[ldst]
        g_of = p_of // 128
        part = p_of % 128
        key = p_of * NW + ew
        eo = np.argsort(key, kind="stable")
        ks = key[eo]
        run_id = np.cumsum(np.r_[0, (np.diff(ks) != 0).astype(np.int64)])
        first = np.r_[0, np.flatnonzero(np.diff(ks)) + 1]
        rank = np.arange(len(ks)) - first[run_id]
        idx_arr = np.full(TOT, DUMMY_LOC, np.int16)
        epos = (gbase[g_of[eo]]
                + (offw[g_of[eo], ew[eo]] + rank) * 128 + part[eo])
        idx_arr[epos] = loc_row[esrc[eo]].astype(np.int16)
        wrapped = idx_arr.reshape(-1, 16).T           # [16, TOT//16]
        idx_tiles.append(np.ascontiguousarray(np.tile(wrapped, (8, 1))))

    return perms, Lgw, offw, Ltot, gbase, TOT, idx_tiles


# ---------------------------------------------------------------------------
# device program
# ---------------------------------------------------------------------------

def _build_program(Lgw, offw, Ltot, gbase, TOT):
    nc = bass.Bass("TRN2", target_bir_lowering=False, debug=False,
                   num_devices=N_CORES, num_swdge_queues=4)

    def dram_in(name, shape, dt=F32):
        return nc.dram_tensor(name, list(shape), dt, kind="ExternalInput").ap()

    xT_d = dram_in("xT", [F_IN, SPT])
    idx_d = dram_in("idx", [128, TOT // 16], I16)
    W1_d = dram_in("W1e", [F_IN, HD1 + 2 * H1])
    W2_d = dram_in("W2e", [HD1, C2 + 2])
    b1_d = dram_in("b1t", [128, HD1])
    b2_d = dram_in("b2t", [128, C2])
    ident_d = dram_in("ident", [128, 128])
    pf1_d = dram_in("padfix1", [SPT - S, 16], BF16)
    pf2_d = dram_in("padfix2", [SPT - S, 1])

    out_shard = nc.dram_tensor("out_shard", [SPT, C2], F32,
                               kind="ExternalOutput").ap()

    maxL = int(Ltot.max())

    with tile.TileContext(nc) as tc:
        with (
            tc.tile_pool(name="dram", bufs=1, space="DRAM") as dram,
            tc.tile_pool(name="const", bufs=1) as constp,
            tc.tile_pool(name="work", bufs=2) as work,
            tc.tile_pool(name="gath", bufs=3) as gath,
            tc.tile_pool(name="psum", bufs=2, space="PSUM") as psum,
        ):
            nc.gpsimd.load_library(library_config.mlp)
            # count registers, one per distinct value: dma_gather's to_reg()
            # allocates a fresh Pool register per call (no caching), and the
            # pool has ~48 free — L quantization keeps the set small. Write-
            # once registers also stay clobber-free across parallel queues.
            _regs = {}
            _q = [0]

            def gather(out_ap, in_ap, idxs_ap, n, elem):
                # round-robin the 4 SWDGE queues: descriptor generation on the
                # Q7 cores parallelizes per queue (measured 9.3 -> 3.0 ns/desc)
                if n not in _regs:
                    _regs[n] = nc.gpsimd.to_reg(n)
                nc.gpsimd.dma_gather(out_ap, in_ap, idxs_ap, n, _regs[n], elem,
                                     single_packet=False,
                                     queue_num=_q[0])
                _q[0] = (_q[0] + 1) % 4

            # ---- persistent DRAM tables -----------------------------------
            t1_shard = dram.tile([SPT, ROW1], BF16)
            t1_full = dram.tile([N_CORES * SPT, ROW1], BF16)
            t2_shard = dram.tile([SPT, ROW2], F32)
            t2_full = dram.tile([N_CORES * SPT, ROW2], F32)
            t1_full[:].tensor.mls.addr_space = "Shared"
            t2_full[:].tensor.mls.addr_space = "Shared"

            # ---- constants into SBUF --------------------------------------
            _cn = [0]
            def const_load(src_ap, shape, dt=F32):
                _cn[0] += 1
                t = constp.tile(shape, dt, tag=f"const{_cn[0]}")
                nc.sync.dma_start(out=t[:], in_=src_ap)
                return t

            W1a = const_load(W1_d[0:128, :], [128, HD1 + 2 * H1])
            W1b = const_load(W1_d[128:256, :], [128, HD1 + 2 * H1])
            W2sb = const_load(W2_d[:, :], [HD1, C2 + 2])
            b1sb = const_load(b1_d[:, :], [128, HD1])
            b2sb = const_load(b2_d[:, :], [128, C2])
            ident = const_load(ident_d[:, :], [128, 128])
            pf1 = const_load(pf1_d[:, :], [SPT - S, 16], BF16)
            pf2 = const_load(pf2_d[:, :], [SPT - S, 1])
            idx_sb = const_load(idx_d[:, :], [128, TOT // 16], I16)
            er1_sb = constp.tile([128, G * H1], F32)
            er2_sb = constp.tile([128, G], F32)

            # ---- node phase: pack layer-1 table rows for own shard --------
            NB = 4  # node groups per xT load / table write batch
            for n in range(G):
                if n % NB == 0:
                    nw_ = min(NB, G - n) * 128
                    cs4 = slice(n * 128, n * 128 + nw_)
                    xa = work.tile([128, NB * 128], F32, tag="xa")
                    xb = work.tile([128, NB * 128], F32, tag="xb")
                    nc.scalar.dma_start(out=xa[:, 0:nw_], in_=xT_d[0:128, cs4])
                    nc.scalar.dma_start(out=xb[:, 0:nw_], in_=xT_d[128:256, cs4])
                    S4 = work.tile([128, NB * ROW1], BF16, tag="S4")
                k = (n % NB) * 128
                p1 = psum.tile([128, HD1 + 2 * H1], F32, tag="p1")
                nc.tensor.matmul(out=p1[:], lhsT=xa[:, k:k + 128], rhs=W1a[:],
                                 start=True, stop=False)
                nc.tensor.matmul(out=p1[:], lhsT=xb[:, k:k + 128], rhs=W1b[:],
                                 start=False, stop=True)
                j = n % NB
                # feat (f32 -> bf16), d-major already via host W1e layout
                nc.scalar.copy(out=S4[:, j * ROW1:j * ROW1 + HD1],
                               in_=p1[:, 0:HD1])
                # ones block for fused denominator
                nc.vector.memset(S4[:, j * ROW1 + HD1:j * ROW1 + HD1 + 8], 1.0)
                # el as raw f32 inside the bf16 row
                S4f = S4[:].bitcast(F32)
                nc.scalar.copy(
                    out=S4f[:, j * 64 + 36:j * 64 + 44],
                    in_=p1[:, HD1:HD1 + H1])
                nc.scalar.copy(out=er1_sb[:, n * H1:(n + 1) * H1],
                               in_=p1[:, HD1 + H1:HD1 + 2 * H1])
                if n % NB == NB - 1 or n == G - 1:
                    m = n % NB + 1
                    a = (n - m + 1) * 128
                    nc.sync.dma_start(
                        out=t1_shard[a:a + m * 128, :]
                            .rearrange("(j p) r -> p j r", p=128),
                        in_=S4[:, 0:m * ROW1]
                            .rearrange("p (j r) -> p j r", r=ROW1))
            # pad rows (S..SPT): el := -3e38 so they act as window dummies
            nc.sync.dma_start(
                out=t1_shard[S:SPT, 72:88], in_=pf1[:, :])

            # ---- AllGather layer-1 table ----------------------------------
            nc.gpsimd.collective_compute(
                "AllGather", OP.bypass,
                replica_groups=[list(range(N_CORES))],
                ins=[t1_shard[:, :].opt()],
                outs=[t1_full[:, :].opt()],
            )

            # ---- layer-1 edge phase (+ fused layer-2 projection) ----------
            for g in range(G):
                Lt = int(Ltot[g])
                F1 = gath.tile([128, maxL * ROW1], BF16, tag="F1")
                for w in range(NW):
                    L = int(Lgw[g, w])
                    o0 = int(offw[g, w])
                    cb = int(gbase[g]) // 16 + o0 * 8
                    gather(
                        F1[:, o0 * ROW1:(o0 + L) * ROW1]
                            .rearrange("p (j e) -> p j e", e=ROW1),
                        t1_full[w * WROWS:(w + 1) * WROWS, :],
                        idx_sb[:, cb:cb + 8 * L],
                        128 * L, ROW1)
                F1v = F1[:, 0:Lt * ROW1].rearrange("p (l r) -> p l r", r=ROW1)
                F1f = F1[:].bitcast(F32)[:, 0:Lt * 64] \
                    .rearrange("p (l c) -> p l c", c=64)
                A = work.tile([128, maxL * H1], F32, tag="A")
                nc.vector.tensor_add(
                    out=A[:, 0:Lt * H1].rearrange("p (l h) -> p l h", h=H1),
                    in0=F1f[:, :, 36:44],
                    in1=er1_sb[:, g * H1:(g + 1) * H1]
                        .rearrange("p (o h) -> p o h", o=1)
                        .to_broadcast([128, Lt, H1]))
                # exp(leaky_relu(x)) == max(exp(x), exp(0.2 x))
                Av = A[:, 0:Lt * H1]
                EXt = work.tile([128, maxL * H1], BF16, tag="EX")
                EXb = work.tile([128, maxL * H1], BF16, tag="EXb")
                EX = EXt[:, 0:Lt * H1]
                nc.scalar.activation(out=EX, in_=Av, func=AF.Exp)
                nc.scalar.activation(out=EXb[:, 0:Lt * H1], in_=Av,
                                     func=AF.Exp, scale=NEG_SLOPE)
                nc.vector.tensor_tensor(out=EX, in0=EX,
                                        in1=EXb[:, 0:Lt * H1], op=OP.max)
                P = work.tile([128, maxL * 80], BF16, tag="P")
                nc.vector.tensor_mul(
                    out=P[:, 0:Lt * 72]
                        .rearrange("p (l j h) -> p l j h", j=9, h=H1),
                    in0=F1v[:, :, 0:72]
                        .rearrange("p l (j h) -> p l j h", h=H1),
                    in1=EX.rearrange("p (l h) -> p l h", h=H1)
                        .rearrange("p l (o h) -> p l o h", o=1)
                        .to_broadcast([128, Lt, 9, H1]))
                U = work.tile([128, 72], F32, tag="U")
                nc.vector.reduce_sum(
                    out=U[:],
                    in_=P[:, 0:Lt * 72].rearrange("p (l c) -> p c l", c=72),
                    axis=AX.X)
                nc.vector.tensor_scalar_max(out=U[:, 64:72], in0=U[:, 64:72],
                                            scalar1=1e-30)
                rinv = work.tile([128, H1], F32, tag="rinv")
                nc.vector.reciprocal(out=rinv[:], in_=U[:, 64:72])
                Ht = work.tile([128, HD1], F32, tag="Ht")
                nc.vector.tensor_mul(
                    out=Ht[:].rearrange("p (j h) -> p j h", h=H1),
                    in0=U[:, 0:64].rearrange("p (j h) -> p j h", h=H1),
                    in1=rinv[:].rearrange("p (o h) -> p o h", o=1)
                        .to_broadcast([128, D1, H1]))
                nc.vector.tensor_add(out=Ht[:], in0=Ht[:], in1=b1sb[:])
                nc.scalar.activation(out=Ht[:], in_=Ht[:], func=AF.Relu)
                # layer-2 projection for these 128 nodes
                pT = psum.tile([HD1, 128], F32, tag="pT")
                nc.tensor.transpose(out=pT[:], in_=Ht[:], identity=ident[:])
                hT = work.tile([HD1, 128], F32, tag="hT")
                nc.scalar.copy(out=hT[:], in_=pT[:])
                p2 = psum.tile([128, C2 + 2], F32, tag="p2")
                nc.tensor.matmul(out=p2[:], lhsT=hT[:], rhs=W2sb[:],
                                 start=True, stop=True)
                S2 = work.tile([128, ROW2], F32, tag="S2")
                nc.scalar.copy(out=S2[:, 0:C2 + 1], in_=p2[:, 0:C2 + 1])
                nc.scalar.copy(out=er2_sb[:, g:g + 1],
                               in_=p2[:, C2 + 1:C2 + 2])
                nc.sync.dma_start(
                    out=t2_shard[g * 128:(g + 1) * 128, :], in_=S2[:])
            # pad rows: el2 := -3e38
            nc.sync.dma_start(
                out=t2_shard[S:SPT, C2:C2 + 1], in_=pf2[:, :])

            # ---- AllGather layer-2 table ----------------------------------
            nc.gpsimd.collective_compute(
                "AllGather", OP.bypass,
                replica_groups=[list(range(N_CORES))],
                ins=[t2_shard[:, :].opt()],
                outs=[t2_full[:, :].opt()],
            )

            # ---- layer-2 edge phase --------------------------------------
            for g in range(G):
                Lt = int(Ltot[g])
                F2t = gath.tile([128, maxL * ROW1], BF16, tag="F1")
                F2 = F2t[:].bitcast(F32)
                for w in range(NW):
                    L = int(Lgw[g, w])
                    o0 = int(offw[g, w])
                    cb = int(gbase[g]) // 16 + o0 * 8
                    gather(
                        F2[:, o0 * ROW2:(o0 + L) * ROW2]
                            .rearrange("p (j e) -> p j e", e=ROW2),
                        t2_full[w * WROWS:(w + 1) * WROWS, :],
                        idx_sb[:, cb:cb + 8 * L],
                        128 * L, ROW2)
                F2v = F2[:, 0:Lt * ROW2].rearrange("p (l r) -> p l r", r=ROW2)
                A2t = work.tile([128, maxL * H1], F32, tag="A")
                A2 = A2t[:, 0:maxL]
                nc.vector.tensor_add(
                    out=A2[:, 0:Lt].rearrange("p (l o) -> p l o", o=1),
                    in0=F2v[:, :, C2:C2 + 1],
                    in1=er2_sb[:, g:g + 1]
                        .rearrange("p (o h) -> p o h", o=1)
                        .to_broadcast([128, Lt, 1]))
                A2v = A2[:, 0:Lt]
                EX2t = work.tile([128, maxL * H1], BF16, tag="EX")
                EX2f = EX2t[:].bitcast(F32)
                EX2 = EX2f[:, 0:maxL]
                EX2b = EX2f[:, maxL:2 * maxL]
                s2t = work.tile([128, 1], F32, tag="s2t")
                nc.scalar.activation(out=EX2[:, 0:Lt], in_=A2v, func=AF.Exp)
                nc.scalar.activation(out=EX2b[:, 0:Lt], in_=A2v,
                                     func=AF.Exp, scale=NEG_SLOPE)
                nc.vector.tensor_tensor(out=EX2[:, 0:Lt], in0=EX2[:, 0:Lt],
                                        in1=EX2b[:, 0:Lt], op=OP.max)
                nc.vector.reduce_sum(out=s2t[:], in_=EX2[:, 0:Lt], axis=AX.X)
                P2t = work.tile([128, maxL * 80], BF16, tag="P")
                P2 = P2t[:].bitcast(F32)[:, 0:maxL * C2]
                nc.vector.tensor_mul(
                    out=P2[:, 0:Lt * C2].rearrange("p (l k) -> p l k", k=C2),
                    in0=F2v[:, :, 0:C2],
                    in1=EX2[:, 0:Lt].rearrange("p (l o) -> p l o", o=1)
                        .to_broadcast([128, Lt, C2]))
                U2 = work.tile([128, C2], F32, tag="U2")
                nc.vector.reduce_sum(
                    out=U2[:],
                    in_=P2[:, 0:Lt * C2].rearrange("p (l k) -> p k l", k=C2),
                    axis=AX.X)
                nc.vector.tensor_scalar_max(out=s2t[:], in0=s2t[:],
                                            scalar1=1e-30)
                rinv2 = work.tile([128, 1], F32, tag="rinv2")
                nc.vector.reciprocal(out=rinv2[:], in_=s2t[:])
                O = work.tile([128, C2], F32, tag="O")
                nc.vector.tensor_mul(
                    out=O[:], in0=U2[:],
                    in1=rinv2[:, 0:1].to_broadcast([128, C2]))
                nc.vector.tensor_add(out=O[:], in0=O[:], in1=b2sb[:])
                nc.sync.dma_start(
                    out=out_shard[g * 128:(g + 1) * 128, :], in_=O[:])

    mybir.codegen_inst_isa_subclasses(nc)
    _split_waits(nc)
    return nc


# ---------------------------------------------------------------------------
# entry point
# ---------------------------------------------------------------------------

def kernel(x, W1, attn_l1, attn_r1, b1, W2, attn_l2, attn_r2, b2, src, dst):
    global LAST_EXEC_NS, LAST_RESULTS
    x = np.asarray(x, np.float32)
    perms, Lgw, offw, Ltot, gbase, TOT, idx_tiles = _host_prep(src, dst)
    nc = _build_program(Lgw, offw, Ltot, gbase, TOT)

    # ---- weights preprocessing (d-major feature order) --------------------
    W1f = np.asarray(W1, np.float32)               # [256, 64] (h-major cols)
    al1 = np.asarray(attn_l1, np.float32).reshape(H1, D1)
    ar1 = np.asarray(attn_r1, np.float32).reshape(H1, D1)
    jh = np.arange(HD1)
    dmaj_from_hmaj = (jh % H1) * D1 + (jh // H1)   # col (d*8+h) <- W1 col h*8+d
    W1j = W1f[:, dmaj_from_hmaj]
    Wl = (W1f.reshape(F_IN, H1, D1) * al1[None]).sum(-1)
    Wr = (W1f.reshape(F_IN, H1, D1) * ar1[None]).sum(-1)
    W1e = np.concatenate([W1j, Wl, Wr], axis=1).astype(np.float32)

    W2f = np.asarray(W2, np.float32)               # [64, 40], rows h-major
    al2 = np.asarray(attn_l2, np.float32).reshape(C2)
    ar2 = np.asarray(attn_r2, np.float32).reshape(C2)
    W2j = W2f.reshape(H1, D1, C2).transpose(1, 0, 2).reshape(HD1, C2)
    W2e = np.concatenate(
        [W2j, (W2j @ al2)[:, None], (W2j @ ar2)[:, None]], axis=1
    ).astype(np.float32)

    b1j = np.asarray(b1, np.float32).reshape(H1, D1).T.reshape(-1)
    b1t = np.tile(b1j.reshape(1, HD1), (128, 1)).astype(np.float32)
    b2t = np.tile(np.asarray(b2, np.float32).reshape(1, C2), (128, 1))

    padfix1 = np.full((SPT - S, H1), NEG_BIG, np.float32) \
        .view(ml_dtypes.bfloat16)                  # [44, 16] raw f32 bytes
    padfix2 = np.full((SPT - S, 1), NEG_BIG, np.float32)

    common = {
        "W1e": W1e, "W2e": W2e, "b1t": b1t, "b2t": b2t.astype(np.float32),
        "ident": np.eye(128, dtype=np.float32),
        "padfix1": padfix1, "padfix2": padfix2,
    }
    in_maps = []
    for c in range(N_CORES):
        xs = np.zeros((F_IN, SPT), np.float32)
        xs[:, 0:S] = x[c * S + perms[c]].T
        in_maps.append({"xT": np.ascontiguousarray(xs),
                        "idx": idx_tiles[c], **common})

    res = run_bass_kernel_spmd(nc, in_maps, core_ids=list(range(N_CORES)),
                               trace=PROFILE)
    LAST_RESULTS = res.results
    LAST_EXEC_NS = res.exec_time_ns
    out = np.zeros((V, C2), np.float32)
    for c in range(N_CORES):
        out[c * S + perms[c]] = res.results[c]["out_shard"][0:S]
    return out


# revision 24
# speedup vs baseline: 1.0222x; 1.0222x over previous
"""GAT 2-layer node classifier on 8 Trainium2 NeuronCores.

Strategy (1D node partitioning + batched SWDGE gathers):
  - dst nodes sharded contiguously across 8 cores (12500 each)
  - host: per core, group 128 dst nodes per "group" (band+argmax profile
    grouping to minimize padding), in-edges packed along the free dim,
    split by 4 source windows (shard pairs, <=25088 rows so dma_gather's
    int16 indices can address them); padded slots point to a pad row
    whose attention logit is -1e30 so exp() contributes 0.
  - device per layer: one dma_gather per (group, window) pulls all edge
    rows (256B each) in a single SWDGE instruction (the previous
    per-edge-slot indirect DMAs paid ~1us of descriptor-generation
    overhead each and dominated the runtime), then softmax-weighted
    aggregation on DVE/Act, layer-2 projection fused into the layer-1
    loop, AllGather of the per-shard gather table between layers.
  - layer-1 table rows: [feat64(bf16, d-major) | ones8(bf16) | el8(f32)]
    = 256B; layer-2 rows: [feat40(f32) | el(f32) | pad] = 256B.
"""

import sys
import types

import numpy as np
import ml_dtypes

# ---------------------------------------------------------------------------
# environment shims (self-contained: only touches in-process state)
# ---------------------------------------------------------------------------


def _ensure_axon_hooks():
    """concourse.bass_utils imports antenv.axon_hooks when tracing under
    axon; some images lack the module. Provide an in-process shim."""
    try:
        import antenv.axon_hooks  # noqa: F401
        return
    except Exception:
        pass
    try:
        import antenv
    except Exception:
        return
    mod = types.ModuleType("antenv.axon_hooks")
    mod._hook = None

    def set_axon_ntff_profile_hook(hook):
        mod._hook = hook

    def get_axon_ntff_profile_hook():
        return mod._hook

    mod.set_axon_ntff_profile_hook = set_axon_ntff_profile_hook
    mod.get_axon_ntff_profile_hook = get_axon_ntff_profile_hook
    sys.modules["antenv.axon_hooks"] = mod
    antenv.axon_hooks = mod


_ensure_axon_hooks()

import concourse.bass as bass          # noqa: E402
import concourse.mybir as mybir        # noqa: E402
import concourse.tile as tile          # noqa: E402
from concourse import library_config   # noqa: E402
from concourse.vector_clock import ScopedClock  # noqa: E402
from concourse.bass_utils import run_bass_kernel_spmd  # noqa: E402

F32 = mybir.dt.float32
BF16 = mybir.dt.bfloat16
I16 = mybir.dt.int16
AF = mybir.ActivationFunctionType
OP = mybir.AluOpType
AX = mybir.AxisListType


def _patched_drain_and_barrier(self, tick_clock, wait_clock):
    # this walrus build rejects multi-wait instructions; emit one wait per
    # nop before the tail drain instead of stacking them on the drain.
    nc = self.nc
    probe = nc.sync.nop(nofuse=True)
    wait_clock.add_sem_waits(probe.ins, ScopedClock({None: tick_clock.global_clock}))
    waits = list(probe.ins.sync_info.on_wait or []) if probe.ins.sync_info else []
    if waits:
        probe.ins.sync_info = mybir.SyncInfo(on_wait=[waits[0]], on_update=[])
        for w in waits[1:]:
            nop = nc.sync.nop(nofuse=True)
            nop.ins.sync_info = mybir.SyncInfo(on_wait=[w], on_update=[])
    nc.sync.drain()
    nc.all_engine_barrier()
    popped = nc._tile_sem_poison_stack.pop()
    assert popped is self._sem_poison
    nc.clear_and_free_semaphores(list(self.sems.allocated().values()))
    nc.all_engine_barrier()


tile.TileContext._drain_and_barrier = _patched_drain_and_barrier


def _split_waits(nc, max_waits=1):
    """Post-pass: any instruction carrying more than max_waits sem-waits gets
    preceding same-engine NoOps carrying the excess."""
    uid = [0]
    for f in nc.m.functions:
        for bb in f.blocks:
            new_insts = []
            for inst in bb.instructions:
                si = getattr(inst, "sync_info", None)
                if si is not None and si.on_wait and len(si.on_wait) > max_waits:
                    waits = list(si.on_wait)
                    excess, keep = waits[:-max_waits], waits[-max_waits:]
                    for i in range(0, len(excess), max_waits):
                        uid[0] += 1
                        new_insts.append(mybir.InstNoOp(
                            name=f"waitsplit-{uid[0]}-{inst.name}",
                            sync_info=mybir.SyncInfo(
                                on_wait=excess[i:i + max_waits], on_update=[]),
                            bass_nofuse=True,
                            engine=inst.engine,
                        ))
                    inst.sync_info = mybir.SyncInfo(
                        on_wait=keep, on_update=list(si.on_update or []))
                new_insts.append(inst)
            bb.instructions = new_insts


# ---------------------------------------------------------------------------
# problem constants (hardcoded per spec)
# ---------------------------------------------------------------------------
N_CORES = 8
V = 100000            # nodes
S = V // N_CORES      # nodes per core shard (12500)
F_IN = 256
H1, D1 = 8, 8         # layer-1 heads x dim
HD1 = H1 * D1         # 64
C2 = 40               # classes (layer-2 single head)
NEG_SLOPE = 0.2
NEG_BIG = -1.0e30
G = (S + 127) // 128  # 98 groups of 128 dst nodes per core
SPT = G * 128         # 12544 table rows per shard
NW = 4                # source windows (shard pairs)
WROWS = 2 * SPT       # 25088 rows per window (< 32768: int16-indexable)
DUMMY_LOC = S         # window-local dummy row (= first pad row of shard 2w)
ROW1 = 128            # layer-1 table row: bf16[128] = 256B
ROW2 = 64             # layer-2 table row: f32[64]  = 256B

# module-level knobs (test harness pokes these)
PROFILE = False
DEBUG = False
LAST_EXEC_NS = None
LAST_RESULTS = None


# ---------------------------------------------------------------------------
# host-side graph preprocessing (integer work only)
# ---------------------------------------------------------------------------

def _host_prep(src, dst):
    src = np.asarray(src).astype(np.int64)
    dst = np.asarray(dst).astype(np.int64)
    order = np.argsort(dst, kind="stable")
    src_s = src[order].astype(np.int64)
    dst_s = dst[order].astype(np.int64)
    bounds = np.searchsorted(dst_s, np.arange(N_CORES + 1) * S)
    wid_all = (src_s // S) >> 1          # source window of each edge

    # per-core grouping: band+argmax profile sort to minimize per-window
    # max-over-partition padding
    perms, cws = [], []
    for c in range(N_CORES):
        sl = slice(bounds[c], bounds[c + 1])
        ldst = dst_s[sl] - c * S
        cw = np.zeros((S, NW), np.int64)
        np.add.at(cw, (ldst, wid_all[sl]), 1)
        deg = cw.sum(1)
        o = np.argsort(-deg, kind="stable")
        band = 1024
        ob = o.copy()
        for b in range(0, S, band):
            idx = o[b:b + band]
            sub = cw[idx]
            amx = np.argmax(sub, 1)
            key2 = -sub[np.arange(len(idx)), amx]
            ob[b:b + band] = idx[np.lexsort((key2, amx))]
        perms.append(ob.astype(np.int64))
        cws.append(cw)

    # table position of every node (within its shard) under final grouping
    pos_all = np.empty(V, np.int64)
    for c in range(N_CORES):
        pos = np.empty(S, np.int64)
        pos[perms[c]] = np.arange(S)
        pos_all[c * S:(c + 1) * S] = pos
    # window-local table row of every node
    loc_row = ((np.arange(V) // S) % 2) * SPT + pos_all   # < 25088

    # shared per-(group, window) column counts: max over cores
    Lgw_c = []
    for c in range(N_CORES):
        cw_perm = cws[c][perms[c]]
        cw_pad = np.vstack([cw_perm, np.zeros((SPT - S, NW), np.int64)])
        Lgw_c.append(cw_pad.reshape(G, 128, NW).max(1))
    Lgw = np.maximum(np.max(np.stack(Lgw_c), axis=0), 1)   # [G, NW], >=1
    # quantize column counts so the distinct set of 128*L immediates fits in
    # the ~48 free Pool registers (to_reg allocates one per distinct value)
    Lgw = np.where(Lgw <= 16, Lgw, ((Lgw + 3) // 4) * 4)
    offw = np.concatenate(
        [np.zeros((G, 1), np.int64), np.cumsum(Lgw, 1)], 1)  # [G, NW+1]
    Ltot = offw[:, -1]
    gbase = np.concatenate([[0], np.cumsum(Ltot * 128)]).astype(np.int64)
    TOT = int(gbase[-1])

    # per-core int16 index arrays in dma_gather wrapped layout
    idx_tiles = []
    for c in range(N_CORES):
        sl = slice(bounds[c], bounds[c + 1])
        ldst = dst_s[sl] - c * S
        esrc = src_s[sl]
        ew = wid_all[sl]
        posn = np.empty(S, np.int64)
        posn[perms[c]] = np.arange(S)
        p_of = posn[ldst]
        g_of = p_of // 128
        part = p_of % 128
        key = p_of * NW + ew
        eo = np.argsort(key, kind="stable")
        ks = key[eo]
        run_id = np.cumsum(np.r_[0, (np.diff(ks) != 0).astype(np.int64)])
        first = np.r_[0, np.flatnonzero(np.diff(ks)) + 1]
        rank = np.arange(len(ks)) - first[run_id]
        idx_arr = np.full(TOT, DUMMY_LOC, np.int16)
        epos = (gbase[g_of[eo]]
                + (offw[g_of[eo], ew[eo]] + rank) * 128 + part[eo])
        idx_arr[epos] = loc_row[esrc[eo]].astype(np.int16)
        wrapped = idx_arr.reshape(-1, 16).T           # [16, TOT//16]
        idx_tiles.append(np.ascontiguousarray(np.tile(wrapped, (8, 1))))

    return perms, Lgw, offw, Ltot, gbase, TOT, idx_tiles


# ---------------------------------------------------------------------------
# device program
# ---------------------------------------------------------------------------

def _build_program(Lgw, offw, Ltot, gbase, TOT):
    nc = bass.Bass("TRN2", target_bir_lowering=False, debug=False,
                   num_devices=N_CORES, num_swdge_queues=4)

    def dram_in(name, shape, dt=F32):
        return nc.dram_tensor(name, list(shape), dt, kind="ExternalInput").ap()

    xT_d = dram_in("xT", [F_IN, SPT])
    idx_d = dram_in("idx", [128, TOT // 16], I16)
    W1_d = dram_in("W1e", [F_IN, HD1 + 2 * H1])
    W2_d = dram_in("W2e", [HD1, C2 + 2])
    b1_d = dram_in("b1t", [128, HD1])
    b2_d = dram_in("b2t", [128, C2])
    ident_d = dram_in("ident", [128, 128])
    pf1_d = dram_in("padfix1", [SPT - S, 16], BF16)
    pf2_d = dram_in("padfix2", [SPT - S, 1])

    out_shard = nc.dram_tensor("out_shard", [SPT, C2], F32,
                               kind="ExternalOutput").ap()

    maxL = int(Ltot.max())

    with tile.TileContext(nc) as tc:
        with (
            tc.tile_pool(name="dram", bufs=1, space="DRAM") as dram,
            tc.tile_pool(name="const", bufs=1) as constp,
            tc.tile_pool(name="work", bufs=2) as work,
            tc.tile_pool(name="gath", bufs=3) as gath,
            tc.tile_pool(name="psum", bufs=2, space="PSUM") as psum,
        ):
            nc.gpsimd.load_library(library_config.mlp)
            # count registers, one per distinct value: dma_gather's to_reg()
            # allocates a fresh Pool register per call (no caching), and the
            # pool has ~48 free — L quantization keeps the set small. Write-
            # once registers also stay clobber-free across parallel queues.
            _regs = {}
            _q = [0]

            def gather(out_ap, in_ap, idxs_ap, n, elem):
                # round-robin the 4 SWDGE queues: descriptor generation on the
                # Q7 cores parallelizes per queue (measured 9.3 -> 3.0 ns/desc)
                if n not in _regs:
                    _regs[n] = nc.gpsimd.to_reg(n)
                nc.gpsimd.dma_gather(out_ap, in_ap, idxs_ap, n, _regs[n], elem,
                                     single_packet=False,
                                     queue_num=_q[0])
                _q[0] = (_q[0] + 1) % 4

            # ---- persistent DRAM tables -----------------------------------
            t1_shard = dram.tile([SPT, ROW1], BF16)
            t1_full = dram.tile([N_CORES * SPT, ROW1], BF16)
            t2_shard = dram.tile([SPT, ROW2], F32)
            t2_full = dram.tile([N_CORES * SPT, ROW2], F32)
            t1_full[:].tensor.mls.addr_space = "Shared"
            t2_full[:].tensor.mls.addr_space = "Shared"

            # ---- constants into SBUF --------------------------------------
            _cn = [0]
            def const_load(src_ap, shape, dt=F32):
                _cn[0] += 1
                t = constp.tile(shape, dt, tag=f"const{_cn[0]}")
                nc.sync.dma_start(out=t[:], in_=src_ap)
                return t

            W1a = const_load(W1_d[0:128, :], [128, HD1 + 2 * H1])
            W1b = const_load(W1_d[128:256, :], [128, HD1 + 2 * H1])
            W2sb = const_load(W2_d[:, :], [HD1, C2 + 2])
            b1sb = const_load(b1_d[:, :], [128, HD1])
            b2sb = const_load(b2_d[:, :], [128, C2])
            ident = const_load(ident_d[:, :], [128, 128])
            pf1 = const_load(pf1_d[:, :], [SPT - S, 16], BF16)
            pf2 = const_load(pf2_d[:, :], [SPT - S, 1])
            idx_sb = const_load(idx_d[:, :], [128, TOT // 16], I16)
            er1_sb = constp.tile([128, G * H1], F32)
            er2_sb = constp.tile([128, G], F32)

            # ---- node phase: pack layer-1 table rows for own shard --------
            NB = 4  # node groups per xT load / table write batch
            for n in range(G):
                if n % NB == 0:
                    nw_ = min(NB, G - n) * 128
                    cs4 = slice(n * 128, n * 128 + nw_)
                    xa = work.tile([128, NB * 128], F32, tag="xa")
                    xb = work.tile([128, NB * 128], F32, tag="xb")
                    nc.scalar.dma_start(out=xa[:, 0:nw_], in_=xT_d[0:128, cs4])
                    nc.scalar.dma_start(out=xb[:, 0:nw_], in_=xT_d[128:256, cs4])
                    S4 = work.tile([128, NB * ROW1], BF16, tag="S4")
                k = (n % NB) * 128
                p1 = psum.tile([128, HD1 + 2 * H1], F32, tag="p1")
                nc.tensor.matmul(out=p1[:], lhsT=xa[:, k:k + 128], rhs=W1a[:],
                                 start=True, stop=False)
                nc.tensor.matmul(out=p1[:], lhsT=xb[:, k:k + 128], rhs=W1b[:],
                                 start=False, stop=True)
                j = n % NB
                # feat (f32 -> bf16), d-major already via host W1e layout
                nc.scalar.copy(out=S4[:, j * ROW1:j * ROW1 + HD1],
                               in_=p1[:, 0:HD1])
                # ones block for fused denominator
                nc.vector.memset(S4[:, j * ROW1 + HD1:j * ROW1 + HD1 + 8], 1.0)
                # el as raw f32 inside the bf16 row
                S4f = S4[:].bitcast(F32)
                nc.vector.tensor_copy(
                    out=S4f[:, j * 64 + 36:j * 64 + 44],
                    in_=p1[:, HD1:HD1 + H1])
                nc.vector.tensor_copy(out=er1_sb[:, n * H1:(n + 1) * H1],
                                      in_=p1[:, HD1 + H1:HD1 + 2 * H1])
                if n % NB == NB - 1 or n == G - 1:
                    m = n % NB + 1
                    a = (n - m + 1) * 128
                    nc.sync.dma_start(
                        out=t1_shard[a:a + m * 128, :]
                            .rearrange("(j p) r -> p j r", p=128),
                        in_=S4[:, 0:m * ROW1]
                            .rearrange("p (j r) -> p j r", r=ROW1))
            # pad rows (S..SPT): el := -1e30 so they act as window dummies
            nc.sync.dma_start(
                out=t1_shard[S:SPT, 72:88], in_=pf1[:, :])

            # ---- AllGather layer-1 table ----------------------------------
            nc.gpsimd.collective_compute(
                "AllGather", OP.bypass,
                replica_groups=[list(range(N_CORES))],
                ins=[t1_shard[:, :].opt()],
                outs=[t1_full[:, :].opt()],
            )

            # ---- layer-1 edge phase (+ fused layer-2 projection) ----------
            for g in range(G):
                Lt = int(Ltot[g])
                F1 = gath.tile([128, maxL * ROW1], BF16, tag="F1")
                for w in range(NW):
                    L = int(Lgw[g, w])
                    o0 = int(offw[g, w])
                    cb = int(gbase[g]) // 16 + o0 * 8
                    gather(
                        F1[:, o0 * ROW1:(o0 + L) * ROW1]
                            .rearrange("p (j e) -> p j e", e=ROW1),
                        t1_full[w * WROWS:(w + 1) * WROWS, :],
                        idx_sb[:, cb:cb + 8 * L],
                        128 * L, ROW1)
                F1v = F1[:, 0:Lt * ROW1].rearrange("p (l r) -> p l r", r=ROW1)
                F1f = F1[:].bitcast(F32)[:, 0:Lt * 64] \
                    .rearrange("p (l c) -> p l c", c=64)
                A = work.tile([128, maxL * H1], F32, tag="A")
                nc.vector.tensor_add(
                    out=A[:, 0:Lt * H1].rearrange("p (l h) -> p l h", h=H1),
                    in0=F1f[:, :, 36:44],
                    in1=er1_sb[:, g * H1:(g + 1) * H1]
                        .rearrange("p (o h) -> p o h", o=1)
                        .to_broadcast([128, Lt, H1]))
                # exp(leaky_relu(x)) == max(exp(x), exp(0.2 x))
                Av = A[:, 0:Lt * H1]
                EXt = work.tile([128, maxL * H1], BF16, tag="EX")
                EXb = work.tile([128, maxL * H1], BF16, tag="EXb")
                EX = EXt[:, 0:Lt * H1]
                nc.scalar.activation(out=EX, in_=Av, func=AF.Exp)
                nc.scalar.activation(out=EXb[:, 0:Lt * H1], in_=Av,
                                     func=AF.Exp, scale=NEG_SLOPE)
                nc.vector.tensor_tensor(out=EX, in0=EX,
                                        in1=EXb[:, 0:Lt * H1], op=OP.max)
                P = work.tile([128, maxL * 80], BF16, tag="P")
                nc.vector.tensor_mul(
                    out=P[:, 0:Lt * 72]
                        .rearrange("p (l j h) -> p l j h", j=9, h=H1),
                    in0=F1v[:, :, 0:72]
                        .rearrange("p l (j h) -> p l j h", h=H1),
                    in1=EX.rearrange("p (l h) -> p l h", h=H1)
                        .rearrange("p l (o h) -> p l o h", o=1)
                        .to_broadcast([128, Lt, 9, H1]))
                U = work.tile([128, 72], F32, tag="U")
                nc.vector.reduce_sum(
                    out=U[:],
                    in_=P[:, 0:Lt * 72].rearrange("p (l c) -> p c l", c=72),
                    axis=AX.X)
                nc.vector.tensor_scalar_max(out=U[:, 64:72], in0=U[:, 64:72],
                                            scalar1=1e-30)
                rinv = work.tile([128, H1], F32, tag="rinv")
                nc.vector.reciprocal(out=rinv[:], in_=U[:, 64:72])
                Ht = work.tile([128, HD1], F32, tag="Ht")
                nc.vector.tensor_mul(
                    out=Ht[:].rearrange("p (j h) -> p j h", h=H1),
                    in0=U[:, 0:64].rearrange("p (j h) -> p j h", h=H1),
                    in1=rinv[:].rearrange("p (o h) -> p o h", o=1)
                        .to_broadcast([128, D1, H1]))
                nc.vector.tensor_add(out=Ht[:], in0=Ht[:], in1=b1sb[:])
                nc.scalar.activation(out=Ht[:], in_=Ht[:], func=AF.Relu)
                # layer-2 projection for these 128 nodes
                pT = psum.tile([HD1, 128], F32, tag="pT")
                nc.tensor.transpose(out=pT[:], in_=Ht[:], identity=ident[:])
                hT = work.tile([HD1, 128], F32, tag="hT")
                nc.scalar.copy(out=hT[:], in_=pT[:])
                p2 = psum.tile([128, C2 + 2], F32, tag="p2")
                nc.tensor.matmul(out=p2[:], lhsT=hT[:], rhs=W2sb[:],
                                 start=True, stop=True)
                S2 = work.tile([128, ROW2], F32, tag="S2")
                nc.scalar.copy(out=S2[:, 0:C2 + 1], in_=p2[:, 0:C2 + 1])
                nc.vector.tensor_copy(out=er2_sb[:, g:g + 1],
                                      in_=p2[:, C2 + 1:C2 + 2])
                nc.sync.dma_start(
                    out=t2_shard[g * 128:(g + 1) * 128, :], in_=S2[:])
            # pad rows: el2 := -1e30
            nc.sync.dma_start(
                out=t2_shard[S:SPT, C2:C2 + 1], in_=pf2[:, :])

            # ---- AllGather layer-2 table ----------------------------------
            nc.gpsimd.collective_compute(
                "AllGather", OP.bypass,
                replica_groups=[list(range(N_CORES))],
                ins=[t2_shard[:, :].opt()],
                outs=[t2_full[:, :].opt()],
            )

            # ---- layer-2 edge phase --------------------------------------
            for g in range(G):
                Lt = int(Ltot[g])
                F2t = gath.tile([128, maxL * ROW1], BF16, tag="F1")
                F2 = F2t[:].bitcast(F32)
                for w in range(NW):
                    L = int(Lgw[g, w])
                    o0 = int(offw[g, w])
                    cb = int(gbase[g]) // 16 + o0 * 8
                    gather(
                        F2[:, o0 * ROW2:(o0 + L) * ROW2]
                            .rearrange("p (j e) -> p j e", e=ROW2),
                        t2_full[w * WROWS:(w + 1) * WROWS, :],
                        idx_sb[:, cb:cb + 8 * L],
                        128 * L, ROW2)
                F2v = F2[:, 0:Lt * ROW2].rearrange("p (l r) -> p l r", r=ROW2)
                A2t = work.tile([128, maxL * H1], F32, tag="A")
                A2 = A2t[:, 0:maxL]
                nc.vector.tensor_add(
                    out=A2[:, 0:Lt].rearrange("p (l o) -> p l o", o=1),
                    in0=F2v[:, :, C2:C2 + 1],
                    in1=er2_sb[:, g:g + 1]
                        .rearrange("p (o h) -> p o h", o=1)
                        .to_broadcast([128, Lt, 1]))
                A2v = A2[:, 0:Lt]
                EX2t = work.tile([128, maxL * H1], BF16, tag="EX")
                EX2f = EX2t[:].bitcast(F32)
                EX2 = EX2f[:, 0:maxL]
                EX2b = EX2f[:, maxL:2 * maxL]
                s2t = work.tile([128, 1], F32, tag="s2t")
                nc.scalar.activation(out=EX2[:, 0:Lt], in_=A2v, func=AF.Exp)
                nc.scalar.activation(out=EX2b[:, 0:Lt], in_=A2v,
                                     func=AF.Exp, scale=NEG_SLOPE)
                nc.vector.tensor_tensor(out=EX2[:, 0:Lt], in0=EX2[:, 0:Lt],
                                        in1=EX2b[:, 0:Lt], op=OP.max)
                nc.vector.reduce_sum(out=s2t[:], in_=EX2[:, 0:Lt], axis=AX.X)
                P2t = work.tile([128, maxL * 80], BF16, tag="P")
                P2 = P2t[:].bitcast(F32)[:, 0:maxL * C2]
                nc.vector.tensor_mul(
                    out=P2[:, 0:Lt * C2].rearrange("p (l k) -> p l k", k=C2),
                    in0=F2v[:, :, 0:C2],
                    in1=EX2[:, 0:Lt].rearrange("p (l o) -> p l o", o=1)
                        .to_broadcast([128, Lt, C2]))
                U2 = work.tile([128, C2], F32, tag="U2")
                nc.vector.reduce_sum(
                    out=U2[:],
                    in_=P2[:, 0:Lt * C2].rearrange("p (l k) -> p k l", k=C2),
                    axis=AX.X)
                nc.vector.tensor_scalar_max(out=s2t[:], in0=s2t[:],
                                            scalar1=1e-30)
                rinv2 = work.tile([128, 1], F32, tag="rinv2")
                nc.vector.reciprocal(out=rinv2[:], in_=s2t[:])
                O = work.tile([128, C2], F32, tag="O")
                nc.vector.tensor_mul(
                    out=O[:], in0=U2[:],
                    in1=rinv2[:, 0:1].to_broadcast([128, C2]))
                nc.vector.tensor_add(out=O[:], in0=O[:], in1=b2sb[:])
                nc.sync.dma_start(
                    out=out_shard[g * 128:(g + 1) * 128, :], in_=O[:])

    mybir.codegen_inst_isa_subclasses(nc)
    _split_waits(nc)
    return nc


# ---------------------------------------------------------------------------
# entry point
# ---------------------------------------------------------------------------

def kernel(x, W1, attn_l1, attn_r1, b1, W2, attn_l2, attn_r2, b2, src, dst):
    global LAST_EXEC_NS, LAST_RESULTS
    x = np.asarray(x, np.float32)
    perms, Lgw, offw, Ltot, gbase, TOT, idx_tiles = _host_prep(src, dst)
    nc = _build_program(Lgw, offw, Ltot, gbase, TOT)

    # ---- weights preprocessing (d-major feature order) --------------------
    W1f = np.asarray(W1, np.float32)               # [256, 64] (h-major cols)
    al1 = np.asarray(attn_l1, np.float32).reshape(H1, D1)
    ar1 = np.asarray(attn_r1, np.float32).reshape(H1, D1)
    jh = np.arange(HD1)
    dmaj_from_hmaj = (jh % H1) * D1 + (jh // H1)   # col (d*8+h) <- W1 col h*8+d
    W1j = W1f[:, dmaj_from_hmaj]
    Wl = (W1f.reshape(F_IN, H1, D1) * al1[None]).sum(-1)
    Wr = (W1f.reshape(F_IN, H1, D1) * ar1[None]).sum(-1)
    W1e = np.concatenate([W1j, Wl, Wr], axis=1).astype(np.float32)

    W2f = np.asarray(W2, np.float32)               # [64, 40], rows h-major
    al2 = np.asarray(attn_l2, np.float32).reshape(C2)
    ar2 = np.asarray(attn_r2, np.float32).reshape(C2)
    W2j = W2f.reshape(H1, D1, C2).transpose(1, 0, 2).reshape(HD1, C2)
    W2e = np.concatenate(
        [W2j, (W2j @ al2)[:, None], (W2j @ ar2)[:, None]], axis=1
    ).astype(np.float32)

    b1j = np.asarray(b1, np.float32).reshape(H1, D1).T.reshape(-1)
    b1t = np.tile(b1j.reshape(1, HD1), (128, 1)).astype(np.float32)
    b2t = np.tile(np.asarray(b2, np.float32).reshape(1, C2), (128, 1))

    padfix1 = np.full((SPT - S, H1), NEG_BIG, np.float32) \
        .view(ml_dtypes.bfloat16)                  # [44, 16] raw f32 bytes
    padfix2 = np.full((SPT - S, 1), NEG_BIG, np.float32)

    common = {
        "W1e": W1e, "W2e": W2e, "b1t": b1t, "b2t": b2t.astype(np.float32),
        "ident": np.eye(128, dtype=np.float32),
        "padfix1": padfix1, "padfix2": padfix2,
    }
    in_maps = []
    for c in range(N_CORES):
        xs = np.zeros((F_IN, SPT), np.float32)
        xs[:, 0:S] = x[c * S + perms[c]].T
        in_maps.append({"xT": np.ascontiguousarray(xs),
                        "idx": idx_tiles[c], **common})

    res = run_bass_kernel_spmd(nc, in_maps, core_ids=list(range(N_CORES)),
                               trace=PROFILE)
    LAST_RESULTS = res.results
    LAST_EXEC_NS = res.exec_time_ns
    out = np.zeros((V, C2), np.float32)
    for c in range(N_CORES):
        out[c * S + perms[c]] = res.results[c]["out_shard"][0:S]
    return out
